# revision 3
# baseline (speedup 1.0000x reference)
"""Low-dim-QK multi-head attention TRN2 Bass kernel (8 NeuronCores).

Reference computation (all fp32):
  Ql = (Q @ Wq.T + bq)  -> (B, TQ, 256) -> heads (B, 8, TQ, 32)
  Kl = (K @ Wk.T + bk)  -> heads (B, 8, TK, 32)
  S  = Ql @ Kl.T / sqrt(32),  masked by key_padding_mask (-inf)
  A  = softmax(S, axis=-1)
  out = concat_h(A_h @ V) @ Wo.T + bo        # V shared across heads

Sharding: (batch, q-chunk) across 8 cores: core c handles batch c//4,
query rows [(c%4)*512, (c%4)*512+512).  No inter-core communication.

On-chip layout (everything transposed so no transposes are ever needed):
  qlT/klT: (r=head*32+hd on partitions, tokens free)  from host-transposed
           Q.T/K.T and Wq.T/Wk.T via out = (W.T).T-style matmuls.
  S.T     per (head, k-tile): matmul(lhsT=klT[32,128], rhs=qlT[32,512]).
  P.T     = exp(S.T * scale + maskbias[k]) on ScalarE (bias is per-partition
           = per-key, which implements key masking for free).
  D       = softmax denominators via ones-column matmuls (PSUM fp32 accum).
  X.T     per (head, d-tile): matmul(lhsT=V[k,d], rhs=P.T) accumulated over
           k-tiles, then normalized by broadcast 1/D on VectorE.
  out     = matmul(lhsT=X.T[c,q], rhs=Wo.T[c,o]) accumulated per head in
           PSUM, then accumulated across heads in SBUF fp32.

Matmuls run in float32r (full PE rate, ~1e-4 relative precision) with fp32
PSUM accumulation.
"""

import numpy as np

import concourse.bass as bass
import concourse.mybir as mybir
import concourse.tile as tile
from concourse import bacc
from concourse.bass_utils import run_bass_kernel_spmd

F32 = mybir.dt.float32
F32R = mybir.dt.float32r
AF = mybir.ActivationFunctionType

B = 2
TQ = 2048
TK = 2048
D = 1024          # model dim of Q/K/V inputs
R = 256           # QK_DIM
H = 8
HD = 32           # head dim of Ql/Kl
DV = 1024         # V dim (shared across heads)
O = 1024          # output dim
NCORES = 8
TQC = TQ * B // NCORES   # 512 query rows per core
KT = TK // 128           # 16 k-tiles
DT = D // 128            # 8 contraction tiles for projections
VT = DV // 128           # 8 d-tiles of V / c-tiles per head
QS = TQC // 128          # 4 query sub-tiles (M=128 each)
SCALE = 1.0 / float(np.sqrt(HD))
MASK_BIAS = -60.0


def _build_kernel(nc: bass.Bass, t):
    qT, kT, v, wqT, wkT, woT, bq, bk, bo, mb, ones_col_d, ones_row_d, out = t
    tc = tile.TileContext(nc)
    with tc, nc.allow_low_precision(
        reason="float32r matmul operands; all accumulation is fp32 in PSUM/SBUF"
    ):
        const = tc.alloc_tile_pool(name="const", bufs=1)

        # ---- persistent SBUF tensors -------------------------------------
        v_sb = const.tile([128, KT, DV], F32R)       # V[k,d]; part = k%128
        klT_sb = const.tile([128, 2, TK], F32R)      # part = r%128, g = r//128
        qlT_sb = const.tile([128, 2, TQC], F32R)
        mb_sb = const.tile([128, KT], F32)           # mask bias per key
        bq_sb = const.tile([128, 2], F32)
        bk_sb = const.tile([128, 2], F32)
        rbo_sb = const.tile([128, O], F32)           # bo replicated over parts
        ones_col = const.tile([128, 1], F32R)
        ones_row = const.tile([1, 128], F32R)
        out_acc = const.tile([128, QS, O], F32)      # part = q%128

        nc.sync.dma_start(out=v_sb, in_=v.ap().rearrange("(kt p) d -> p kt d", p=128))
        nc.sync.dma_start(out=mb_sb, in_=mb.ap().rearrange("(kt p) -> p kt", p=128))
        nc.sync.dma_start(out=bq_sb, in_=bq.ap().rearrange("(g p) -> p g", p=128))
        nc.sync.dma_start(out=bk_sb, in_=bk.ap().rearrange("(g p) -> p g", p=128))
        nc.sync.dma_start(out=ones_col, in_=ones_col_d.ap())
        nc.sync.dma_start(out=ones_row, in_=ones_row_d.ap())
        bo_bcast = bass.AP(
            tensor=bo.ap().tensor, offset=0, ap=[[0, 128], [1, O]]
        )
        nc.sync.dma_start(out=rbo_sb, in_=bo_bcast)

        # ---- phase A: Q/K low-dim projections ----------------------------
        with tc.tile_pool(name="proj", bufs=1) as proj, \
             tc.tile_pool(name="projk", bufs=2) as projk, \
             tc.tile_pool(name="ps_a", bufs=2, space="PSUM") as ps_a:
            wq_sb = proj.tile([128, DT, R], F32R)
            wk_sb = proj.tile([128, DT, R], F32R)
            qt_sb = proj.tile([128, DT, TQC], F32R)
            nc.sync.dma_start(
                out=wq_sb, in_=wqT.ap().rearrange("(dt p) r -> p dt r", p=128)
            )
            nc.sync.dma_start(
                out=wk_sb, in_=wkT.ap().rearrange("(dt p) r -> p dt r", p=128)
            )
            nc.sync.dma_start(
                out=qt_sb, in_=qT.ap().rearrange("(dt p) q -> p dt q", p=128)
            )

            # Ql.T = Wq @ Q.T: lhsT = Wq.T (d, r) tiles, rhs = Q.T (d, q)
            for g in range(2):
                ql_ps = ps_a.tile([128, TQC], F32, tag="psa")
                for dt_ in range(DT):
                    nc.tensor.matmul(
                        ql_ps,
                        lhsT=wq_sb[:, dt_, g * 128 : (g + 1) * 128],
                        rhs=qt_sb[:, dt_, :],
                        start=(dt_ == 0),
                        stop=(dt_ == DT - 1),
                    )
                nc.scalar.activation(
                    out=qlT_sb[:, g, :], in_=ql_ps, func=AF.Identity,
                    bias=bq_sb[:, g : g + 1], scale=1.0,
                )

            # Kl.T, processed in 512-token chunks so DMA overlaps compute
            for nt in range(TK // 512):
                ktc_sb = projk.tile([128, DT, 512], F32R, tag="ktc")
                nc.sync.dma_start(
                    out=ktc_sb,
                    in_=kT.ap()[:, nt * 512 : (nt + 1) * 512].rearrange(
                        "(dt p) t -> p dt t", p=128
                    ),
                )
                for g in range(2):
                    kl_ps = ps_a.tile([128, 512], F32, tag="psa")
                    for dt_ in range(DT):
                        nc.tensor.matmul(
                            kl_ps,
                            lhsT=wk_sb[:, dt_, g * 128 : (g + 1) * 128],
                            rhs=ktc_sb[:, dt_, :],
                            start=(dt_ == 0),
                            stop=(dt_ == DT - 1),
                        )
                    nc.scalar.activation(
                        out=klT_sb[:, g, nt * 512 : (nt + 1) * 512], in_=kl_ps,
                        func=AF.Identity, bias=bk_sb[:, g : g + 1], scale=1.0,
                    )

        # ---- phase B: per-head attention + fused out-projection ----------
        with tc.tile_pool(name="pt", bufs=17) as pt_pool, \
             tc.tile_pool(name="xt", bufs=10) as xt_pool, \
             tc.tile_pool(name="wo", bufs=2) as wo_pool, \
             tc.tile_pool(name="smalls", bufs=2) as smalls, \
             tc.tile_pool(name="ps_s", bufs=2, space="PSUM") as ps_s, \
             tc.tile_pool(name="ps_d", bufs=2, space="PSUM") as ps_d, \
             tc.tile_pool(name="ps_o", bufs=2, space="PSUM") as ps_o, \
             tc.tile_pool(name="ps_p", bufs=2, space="PSUM") as ps_p:
            for h in range(H):
                g, pb = divmod(h, 4)
                pb *= 32
                # scores.T -> exp -> P.T, with denominator + first d-tile of
                # attnV interleaved per k-tile so PE work paces with ScalarE
                pts = []
                d_ps = ps_d.tile([1, TQC], F32, tag="d")
                o_ps0 = ps_o.tile([128, TQC], F32, tag="o")
                for kt_ in range(KT):
                    st = ps_s.tile([128, TQC], F32, tag="st")
                    nc.tensor.matmul(
                        st,
                        lhsT=klT_sb[pb : pb + 32, g, kt_ * 128 : (kt_ + 1) * 128],
                        rhs=qlT_sb[pb : pb + 32, g, :],
                        start=True,
                        stop=True,
                        tile_position=(pb, 0),
                    )
                    pt = pt_pool.tile([128, TQC], F32R, tag="pt")
                    nc.scalar.activation(
                        out=pt, in_=st, func=AF.Exp,
                        bias=mb_sb[:, kt_ : kt_ + 1], scale=SCALE,
                    )
                    pts.append(pt)
                    nc.tensor.matmul(
                        d_ps, lhsT=ones_col, rhs=pt,
                        start=(kt_ == 0), stop=(kt_ == KT - 1),
                    )
                    nc.tensor.matmul(
                        o_ps0, lhsT=v_sb[:, kt_, 0:128], rhs=pt,
                        start=(kt_ == 0), stop=(kt_ == KT - 1),
                    )

                # 1/D broadcast to all partitions (via ones ⊗ recip matmul)
                d_sb = smalls.tile([1, TQC], F32R, tag="dsb")
                nc.vector.reciprocal(out=d_sb, in_=d_ps)
                rep_ps = ps_s.tile([128, TQC], F32, tag="st")
                nc.tensor.matmul(rep_ps, lhsT=ones_row, rhs=d_sb, start=True, stop=True)
                rep_sb = smalls.tile([128, TQC], F32, tag="rep")
                nc.scalar.copy(out=rep_sb, in_=rep_ps)

                # attnV remaining d-tiles: X.T[d,q] = sum_k V[k,d] P.T[k,q]
                xts = []
                for dt_ in range(VT):
                    if dt_ == 0:
                        o_ps = o_ps0
                    else:
                        o_ps = ps_o.tile([128, TQC], F32, tag="o")
                        for kt_ in range(KT):
                            nc.tensor.matmul(
                                o_ps,
                                lhsT=v_sb[:, kt_, dt_ * 128 : (dt_ + 1) * 128],
                                rhs=pts[kt_],
                                start=(kt_ == 0),
                                stop=(kt_ == KT - 1),
                            )
                    xt = xt_pool.tile([128, TQC], F32R, tag="xt")
                    nc.vector.tensor_mul(xt, o_ps, rep_sb)
                    xts.append(xt)

                # out-projection contribution of this head:
                # out[q,o] += sum_{c in head} X.T[c,q].T @ Wo.T[c,o]
                for oj in range(2):
                    wo_sb = wo_pool.tile([128, VT, 512], F32R, tag="wo")
                    nc.sync.dma_start(
                        out=wo_sb,
                        in_=woT.ap()[
                            h * DV : (h + 1) * DV, oj * 512 : (oj + 1) * 512
                        ].rearrange("(ct p) o -> p ct o", p=128),
                    )
                    for qs_ in range(QS):
                        op_ps = ps_p.tile([128, 512], F32, tag="op")
                        for ct in range(VT):
                            nc.tensor.matmul(
                                op_ps,
                                lhsT=xts[ct][:, qs_ * 128 : (qs_ + 1) * 128],
                                rhs=wo_sb[:, ct, :],
                                start=(ct == 0),
                                stop=(ct == VT - 1),
                            )
                        acc = out_acc[:, qs_, oj * 512 : (oj + 1) * 512]
                        if h == 0:
                            nc.vector.tensor_add(
                                acc, op_ps, rbo_sb[:, oj * 512 : (oj + 1) * 512]
                            )
                        else:
                            nc.vector.tensor_add(acc, op_ps, acc)

        nc.sync.dma_start(
            out=out.ap().rearrange("(qs p) o -> p qs o", p=128), in_=out_acc
        )
        const.release()


_NC_CACHE = None


def _get_nc():
    global _NC_CACHE
    if _NC_CACHE is not None:
        return _NC_CACHE
    nc = bacc.Bacc("TRN2", target_bir_lowering=False, debug=False,
                   num_devices=NCORES)
    qT = nc.dram_tensor("qT", (D, TQC), F32R, kind="ExternalInput")
    kT = nc.dram_tensor("kT", (D, TK), F32R, kind="ExternalInput")
    v = nc.dram_tensor("v", (TK, DV), F32R, kind="ExternalInput")
    wqT = nc.dram_tensor("wqT", (D, R), F32R, kind="ExternalInput")
    wkT = nc.dram_tensor("wkT", (D, R), F32R, kind="ExternalInput")
    woT = nc.dram_tensor("woT", (H * DV, O), F32R, kind="ExternalInput")
    bq = nc.dram_tensor("bq", (R,), F32, kind="ExternalInput")
    bk = nc.dram_tensor("bk", (R,), F32, kind="ExternalInput")
    bo = nc.dram_tensor("bo", (O,), F32, kind="ExternalInput")
    mb = nc.dram_tensor("mb", (TK,), F32, kind="ExternalInput")
    ones_col_d = nc.dram_tensor("ones_col_d", (128, 1), F32R, kind="ExternalInput")
    ones_row_d = nc.dram_tensor("ones_row_d", (1, 128), F32R, kind="ExternalInput")
    out = nc.dram_tensor("out", (TQC, O), F32, kind="ExternalOutput")
    _build_kernel(
        nc, (qT, kT, v, wqT, wkT, woT, bq, bk, bo, mb, ones_col_d, ones_row_d, out)
    )
    nc.compile()
    _NC_CACHE = nc
    return nc


def _prep_in_maps(Q, K, V, Wq, bq, Wk, bk, Wo, bo, key_padding_mask):
    Q = np.asarray(Q, dtype=np.float32)
    K = np.asarray(K, dtype=np.float32)
    V = np.asarray(V, dtype=np.float32)
    Wq = np.asarray(Wq, dtype=np.float32)
    Wk = np.asarray(Wk, dtype=np.float32)
    Wo = np.asarray(Wo, dtype=np.float32)
    bq = np.ascontiguousarray(np.asarray(bq, dtype=np.float32))
    bk = np.ascontiguousarray(np.asarray(bk, dtype=np.float32))
    bo = np.ascontiguousarray(np.asarray(bo, dtype=np.float32))
    mask = np.asarray(key_padding_mask)

    wqT = np.ascontiguousarray(Wq.T)
    wkT = np.ascontiguousarray(Wk.T)
    woT = np.ascontiguousarray(Wo.T)
    ones_col = np.ones((128, 1), np.float32)
    ones_row = np.ones((1, 128), np.float32)
    kT = [np.ascontiguousarray(K[b].T) for b in range(B)]
    vb = [np.ascontiguousarray(V[b]) for b in range(B)]
    mb = [
        np.where(mask[b], np.float32(MASK_BIAS), np.float32(0.0)).astype(np.float32)
        for b in range(B)
    ]

    in_maps = []
    for c in range(NCORES):
        b, chunk = divmod(c, NCORES // B)
        q0 = chunk * TQC
        in_maps.append(
            {
                "qT": np.ascontiguousarray(Q[b, q0 : q0 + TQC, :].T),
                "kT": kT[b],
                "v": vb[b],
                "wqT": wqT,
                "wkT": wkT,
                "woT": woT,
                "bq": bq,
                "bk": bk,
                "bo": bo,
                "mb": mb[b],
                "ones_col_d": ones_col,
                "ones_row_d": ones_row,
            }
        )
    return in_maps


def run(inputs: dict, **spmd_kwargs):
    """Build (cached), run on 8 cores, return (full_output, BassKernelResults)."""
    nc = _get_nc()
    in_maps = _prep_in_maps(**inputs)
    res = run_bass_kernel_spmd(nc, in_maps, core_ids=list(range(NCORES)),
                               **spmd_kwargs)
    out = np.empty((B, TQ, O), np.float32)
    for c in range(NCORES):
        b, chunk = divmod(c, NCORES // B)
        q0 = chunk * TQC
        out[b, q0 : q0 + TQC, :] = res.results[c]["out"]
    return out, res


def kernel(**inputs) -> np.ndarray:
    out, _ = run(inputs)
    return out


# revision 5
# speedup vs baseline: 8.4511x; 8.4511x over previous
"""Low-dim-QK multi-head attention TRN2 Bass kernel (8 NeuronCores).

Reference computation (all fp32):
  Ql = (Q @ Wq.T + bq)  -> (B, TQ, 256) -> heads (B, 8, TQ, 32)
  Kl = (K @ Wk.T + bk)  -> heads (B, 8, TK, 32)
  S  = Ql @ Kl.T / sqrt(32),  masked by key_padding_mask (-inf)
  A  = softmax(S, axis=-1)
  out = concat_h(A_h @ V) @ Wo.T + bo        # V shared across heads

Sharding: (batch, q-chunk) across 8 cores: core c handles batch c//4,
query rows [(c%4)*512, (c%4)*512+512).  No inter-core communication.

On-chip layout (everything transposed so no transposes are ever needed):
  qlT/klT: (r=head*32+hd on partitions, tokens free)  from host-transposed
           Q.T/K.T and Wq.T/Wk.T via out = Wq @ Q.T-style matmuls.
  S.T     per (head, k-tile): matmul(lhsT=klT[32,128], rhs=qlT[32,512]).
  P.T     = exp(S.T * scale + maskbias[k]) on ScalarE (bias is per-partition
           = per-key, which implements key masking for free).
  D       = softmax denominators via ones-column matmuls (PSUM fp32 accum).
  X.T     per (head, d-tile): matmul(lhsT=V[k,d], rhs=P.T) accumulated over
           k-tiles, then normalized by broadcast 1/D on VectorE.
  out     = matmul(lhsT=X.T[c,q], rhs=Wo.T[c,o]) accumulated per head in
           PSUM, then accumulated across heads in SBUF fp32.

Matmuls run in float32r (full PE rate, ~1e-4 relative precision) with fp32
PSUM accumulation.
"""

import numpy as np

import concourse.bass as bass
import concourse.mybir as mybir
import concourse.tile as tile
from concourse import bacc
from concourse.bass_utils import run_bass_kernel_spmd

F32 = mybir.dt.float32
F32R = mybir.dt.float32r
AF = mybir.ActivationFunctionType

B = 2
TQ = 2048
TK = 2048
D = 1024          # model dim of Q/K/V inputs
R = 256           # QK_DIM
H = 8
HD = 32           # head dim of Ql/Kl
DV = 1024         # V dim (shared across heads)
O = 1024          # output dim
NCORES = 8
TQC = TQ * B // NCORES   # 512 query rows per core
KT = TK // 128           # 16 k-tiles
DT = D // 128            # 8 contraction tiles for projections
VT = DV // 128           # 8 d-tiles of V / c-tiles per head
QS = TQC // 128          # 4 query sub-tiles (M=128 each)
SCALE = 1.0 / float(np.sqrt(HD))
MASK_BIAS = -60.0


def _body(nc, t, pools):
    qT, kT, v, wqT, wkT, woT, bq, bk, bo, mb, ones_col_d, ones_row_d, out = t
    (const, wpool, pt_pool, xt_pool, wo_pool, smalls,
     ps_s, ps_d, ps_o, ps_p) = pools

    # ---- persistent SBUF tensors -------------------------------------
    v_sb = const.tile([128, KT, DV], F32R, tag="v")       # V[k,d]; part = k%128
    klT_sb = const.tile([128, 2, TK], F32R, tag="klT")    # part = r%128, g = r//128
    qlT_sb = const.tile([128, 2, TQC], F32R, tag="qlT")
    mb_sb = const.tile([128, KT], F32, tag="mb")          # mask bias per key
    bq_sb = const.tile([128, 2], F32, tag="bq")
    bk_sb = const.tile([128, 2], F32, tag="bk")
    rbo_sb = const.tile([128, O], F32, tag="rbo")         # bo replicated over parts
    ones_col = const.tile([128, 1], F32R, tag="onc")
    ones_row = const.tile([1, 128], F32R, tag="onr")
    out_acc = const.tile([128, QS, O], F32, tag="oacc")   # part = q%128

    nc.sync.dma_start(out=v_sb, in_=v.ap().rearrange("(kt p) d -> p kt d", p=128))
    nc.sync.dma_start(out=mb_sb, in_=mb.ap().rearrange("(kt p) -> p kt", p=128))
    nc.sync.dma_start(out=bq_sb, in_=bq.ap().rearrange("(g p) -> p g", p=128))
    nc.sync.dma_start(out=bk_sb, in_=bk.ap().rearrange("(g p) -> p g", p=128))
    nc.sync.dma_start(out=ones_col, in_=ones_col_d.ap())
    nc.sync.dma_start(out=ones_row, in_=ones_row_d.ap())
    bo_bcast = bass.AP(tensor=bo.ap().tensor, offset=0, ap=[[0, 128], [1, O]])
    nc.sync.dma_start(out=rbo_sb, in_=bo_bcast)

    # ---- phase A: Q/K low-dim projections ----------------------------
    wq_sb = wpool.tile([128, DT, R], F32R, tag="wq")
    wk_sb = wpool.tile([128, DT, R], F32R, tag="wk")
    nc.sync.dma_start(out=wq_sb, in_=wqT.ap().rearrange("(dt p) r -> p dt r", p=128))
    nc.sync.dma_start(out=wk_sb, in_=wkT.ap().rearrange("(dt p) r -> p dt r", p=128))

    # Ql.T = Wq @ Q.T: contraction-chunk streaming through the pt pool,
    # both r-groups (g) accumulate in parallel PSUM banks.
    ql_ps = [ps_s.tile([128, TQC], F32, tag="st", name=f"qlps{g}") for g in range(2)]
    for dt_ in range(DT):
        qc = pt_pool.tile([128, TQC], F32R, tag="pt")
        nc.sync.dma_start(
            out=qc,
            in_=qT.ap()[dt_ * 128 : (dt_ + 1) * 128, :],
        )
        for g in range(2):
            nc.tensor.matmul(
                ql_ps[g],
                lhsT=wq_sb[:, dt_, g * 128 : (g + 1) * 128],
                rhs=qc,
                start=(dt_ == 0),
                stop=(dt_ == DT - 1),
            )
    for g in range(2):
        nc.scalar.activation(
            out=qlT_sb[:, g, :], in_=ql_ps[g], func=AF.Identity,
            bias=bq_sb[:, g : g + 1], scale=1.0,
        )

    # Kl.T in 512-token chunks
    for nt in range(TK // 512):
        kl_ps = [ps_s.tile([128, 512], F32, tag="st", name=f"klps{nt}_{g}") for g in range(2)]
        for dt_ in range(DT):
            kc = pt_pool.tile([128, 512], F32R, tag="pt")
            nc.sync.dma_start(
                out=kc,
                in_=kT.ap()[dt_ * 128 : (dt_ + 1) * 128,
                            nt * 512 : (nt + 1) * 512],
            )
            for g in range(2):
                nc.tensor.matmul(
                    kl_ps[g],
                    lhsT=wk_sb[:, dt_, g * 128 : (g + 1) * 128],
                    rhs=kc,
                    start=(dt_ == 0),
                    stop=(dt_ == DT - 1),
                )
        for g in range(2):
            nc.scalar.activation(
                out=klT_sb[:, g, nt * 512 : (nt + 1) * 512], in_=kl_ps[g],
                func=AF.Identity, bias=bk_sb[:, g : g + 1], scale=1.0,
            )

    # ---- phase B: per-head attention + fused out-projection ----------
    for h in range(H):
        g, pb = divmod(h, 4)
        pb *= 32
        # scores.T -> exp -> P.T, with denominator + first d-tile of
        # attnV interleaved per k-tile so PE work paces with ScalarE
        pts = []
        d_ps = ps_d.tile([1, TQC], F32, tag="d")
        o_ps0 = ps_o.tile([128, TQC], F32, tag="o")
        for kt_ in range(KT):
            st = ps_s.tile([128, TQC], F32, tag="st")
            nc.tensor.matmul(
                st,
                lhsT=klT_sb[pb : pb + 32, g, kt_ * 128 : (kt_ + 1) * 128],
                rhs=qlT_sb[pb : pb + 32, g, :],
                start=True,
                stop=True,
                tile_position=(pb, 0),
            )
            pt = pt_pool.tile([128, TQC], F32R, tag="pt")
            nc.scalar.activation(
                out=pt, in_=st, func=AF.Exp,
                bias=mb_sb[:, kt_ : kt_ + 1], scale=SCALE,
            )
            pts.append(pt)
            nc.tensor.matmul(
                d_ps, lhsT=ones_col, rhs=pt,
                start=(kt_ == 0), stop=(kt_ == KT - 1),
            )
            nc.tensor.matmul(
                o_ps0, lhsT=v_sb[:, kt_, 0:128], rhs=pt,
                start=(kt_ == 0), stop=(kt_ == KT - 1),
            )

        # 1/D broadcast to all partitions (via ones ⊗ recip matmul)
        d_sb = smalls.tile([1, TQC], F32R, tag="dsb")
        nc.vector.reciprocal(out=d_sb, in_=d_ps)
        rep_ps = ps_s.tile([128, TQC], F32, tag="st")
        nc.tensor.matmul(rep_ps, lhsT=ones_row, rhs=d_sb, start=True, stop=True)
        rep_sb = smalls.tile([128, TQC], F32, tag="rep")
        nc.scalar.copy(out=rep_sb, in_=rep_ps)

        # attnV remaining d-tiles: X.T[d,q] = sum_k V[k,d] P.T[k,q]
        xts = []
        for dt_ in range(VT):
            if dt_ == 0:
                o_ps = o_ps0
            else:
                o_ps = ps_o.tile([128, TQC], F32, tag="o")
                for kt_ in range(KT):
                    nc.tensor.matmul(
                        o_ps,
                        lhsT=v_sb[:, kt_, dt_ * 128 : (dt_ + 1) * 128],
                        rhs=pts[kt_],
                        start=(kt_ == 0),
                        stop=(kt_ == KT - 1),
                    )
            xt = xt_pool.tile([128, TQC], F32R, tag="xt")
            nc.vector.tensor_mul(xt, o_ps, rep_sb)
            xts.append(xt)

        # out-projection contribution of this head:
        # out[q,o] += sum_{c in head} X.T[c,q].T @ Wo.T[c,o]
        for oj in range(2):
            wos = []
            for cp in range(VT // 2):
                wo_sb = wo_pool.tile([128, 2, 512], F32R, tag="wo")
                nc.sync.dma_start(
                    out=wo_sb,
                    in_=woT.ap()[
                        h * DV + cp * 256 : h * DV + (cp + 1) * 256,
                        oj * 512 : (oj + 1) * 512,
                    ].rearrange("(c p) o -> p c o", p=128),
                )
                wos.append(wo_sb)
            for qs_ in range(QS):
                op_ps = ps_p.tile([128, 512], F32, tag="op")
                for ct in range(VT):
                    nc.tensor.matmul(
                        op_ps,
                        lhsT=xts[ct][:, qs_ * 128 : (qs_ + 1) * 128],
                        rhs=wos[ct // 2][:, ct % 2, :],
                        start=(ct == 0),
                        stop=(ct == VT - 1),
                    )
                acc = out_acc[:, qs_, oj * 512 : (oj + 1) * 512]
                if h == 0:
                    nc.vector.tensor_add(
                        acc, op_ps, rbo_sb[:, oj * 512 : (oj + 1) * 512]
                    )
                else:
                    nc.vector.tensor_add(acc, op_ps, acc)

    nc.sync.dma_start(
        out=out.ap().rearrange("(qs p) o -> p qs o", p=128), in_=out_acc
    )


def _build_kernel(nc, t, loop_n=None):
    tc = tile.TileContext(nc)
    with tc, nc.allow_low_precision(
        reason="float32r matmul operands; all accumulation is fp32 in PSUM/SBUF"
    ):
        pools = (
            tc.alloc_tile_pool(name="const", bufs=1),
            tc.alloc_tile_pool(name="wpool", bufs=1),
            tc.alloc_tile_pool(name="pt", bufs=16),
            tc.alloc_tile_pool(name="xt", bufs=9),
            tc.alloc_tile_pool(name="wo", bufs=5),
            tc.alloc_tile_pool(name="smalls", bufs=2),
            tc.alloc_tile_pool(name="ps_s", bufs=2, space="PSUM"),
            tc.alloc_tile_pool(name="ps_d", bufs=2, space="PSUM"),
            tc.alloc_tile_pool(name="ps_o", bufs=2, space="PSUM"),
            tc.alloc_tile_pool(name="ps_p", bufs=2, space="PSUM"),
        )
        if loop_n is None:
            _body(nc, t, pools)
        else:
            with tc.For_i(0, loop_n, 1):
                _body(nc, t, pools)
        for p in reversed(pools):
            p.release()
    nc.compile()


def _declare_io(nc):
    qT = nc.dram_tensor("qT", (D, TQC), F32R, kind="ExternalInput")
    kT = nc.dram_tensor("kT", (D, TK), F32R, kind="ExternalInput")
    v = nc.dram_tensor("v", (TK, DV), F32R, kind="ExternalInput")
    wqT = nc.dram_tensor("wqT", (D, R), F32R, kind="ExternalInput")
    wkT = nc.dram_tensor("wkT", (D, R), F32R, kind="ExternalInput")
    woT = nc.dram_tensor("woT", (H * DV, O), F32R, kind="ExternalInput")
    bq = nc.dram_tensor("bq", (R,), F32, kind="ExternalInput")
    bk = nc.dram_tensor("bk", (R,), F32, kind="ExternalInput")
    bo = nc.dram_tensor("bo", (O,), F32, kind="ExternalInput")
    mb = nc.dram_tensor("mb", (TK,), F32, kind="ExternalInput")
    ones_col_d = nc.dram_tensor("ones_col_d", (128, 1), F32R, kind="ExternalInput")
    ones_row_d = nc.dram_tensor("ones_row_d", (1, 128), F32R, kind="ExternalInput")
    out = nc.dram_tensor("out", (TQC, O), F32, kind="ExternalOutput")
    return (qT, kT, v, wqT, wkT, woT, bq, bk, bo, mb, ones_col_d, ones_row_d, out)


def build_nc(loop_n=None):
    nc = bacc.Bacc("TRN2", target_bir_lowering=False, debug=False,
                   num_devices=NCORES)
    t = _declare_io(nc)
    _build_kernel(nc, t, loop_n=loop_n)
    return nc


_NC_CACHE = None


def _get_nc():
    global _NC_CACHE
    if _NC_CACHE is None:
        _NC_CACHE = build_nc()
    return _NC_CACHE


def _prep_in_maps(Q, K, V, Wq, bq, Wk, bk, Wo, bo, key_padding_mask):
    Q = np.asarray(Q, dtype=np.float32)
    K = np.asarray(K, dtype=np.float32)
    V = np.asarray(V, dtype=np.float32)
    Wq = np.asarray(Wq, dtype=np.float32)
    Wk = np.asarray(Wk, dtype=np.float32)
    Wo = np.asarray(Wo, dtype=np.float32)
    bq = np.ascontiguousarray(np.asarray(bq, dtype=np.float32))
    bk = np.ascontiguousarray(np.asarray(bk, dtype=np.float32))
    bo = np.ascontiguousarray(np.asarray(bo, dtype=np.float32))
    mask = np.asarray(key_padding_mask)

    wqT = np.ascontiguousarray(Wq.T)
    wkT = np.ascontiguousarray(Wk.T)
    woT = np.ascontiguousarray(Wo.T)
    ones_col = np.ones((128, 1), np.float32)
    ones_row = np.ones((1, 128), np.float32)
    kT = [np.ascontiguousarray(K[b].T) for b in range(B)]
    vb = [np.ascontiguousarray(V[b]) for b in range(B)]
    mb = [
        np.where(mask[b], np.float32(MASK_BIAS), np.float32(0.0)).astype(np.float32)
        for b in range(B)
    ]

    in_maps = []
    for c in range(NCORES):
        b, chunk = divmod(c, NCORES // B)
        q0 = chunk * TQC
        in_maps.append(
            {
                "qT": np.ascontiguousarray(Q[b, q0 : q0 + TQC, :].T),
                "kT": kT[b],
                "v": vb[b],
                "wqT": wqT,
                "wkT": wkT,
                "woT": woT,
                "bq": bq,
                "bk": bk,
                "bo": bo,
                "mb": mb[b],
                "ones_col_d": ones_col,
                "ones_row_d": ones_row,
            }
        )
    return in_maps


def run(inputs: dict, **spmd_kwargs):
    """Build (cached), run on 8 cores, return (full_output, BassKernelResults)."""
    nc = _get_nc()
    in_maps = _prep_in_maps(**inputs)
    res = run_bass_kernel_spmd(nc, in_maps, core_ids=list(range(NCORES)),
                               **spmd_kwargs)
    out = np.empty((B, TQ, O), np.float32)
    for c in range(NCORES):
        b, chunk = divmod(c, NCORES // B)
        q0 = chunk * TQC
        out[b, q0 : q0 + TQC, :] = res.results[c]["out"]
    return out, res


def kernel(**inputs) -> np.ndarray:
    out, _ = run(inputs)
    return out
